# revision 8
# baseline (speedup 1.0000x reference)
"""Causal GQA attention (B=4, S=2048, H=16, KH=4, D=128) on 8 TRN2 NeuronCores.

Sharding: tensor-parallel over heads — each core owns 2 query heads and the
matching KV head; per core that is 8 independent (batch, head) causal
attention jobs of S=2048, D=128.

Per-core kernel (Bass/Tile), per job:
  - scores computed transposed, sT[k, q] = K_tile^T-stationary matmul against
    pre-transposed Q (f32r, full PE rate at N>=256)
  - softmax without running max (scores bounded: |SCALE * qk| < ~10), so
    P = exp(SCALE * sT) directly on ScalarE (PSUM -> SBUF bf16); full
    (below-diagonal) k-tiles are exp'd two at a time from a 2-bank PSUM
    region to amortize ScalarE instruction overhead
  - causal mask applied post-exp as a bf16 triangular multiply on the
    diagonal 128x128 tile only
  - PV: P-tile-stationary matmul against V augmented with a ones column;
    PSUM accumulates both the output numerator and the softmax denominator
    across k tiles; two q-tiles' accumulators share one PSUM bank.
    Final normalize = batched reciprocal + broadcast multiply on DVE.
"""

import numpy as np
import ml_dtypes

import concourse.bass as bass
import concourse.mybir as mybir
import concourse.tile as tile
from concourse import bacc
from concourse.bass_utils import run_bass_kernel_spmd

P = 128
B, S, H, KH, D = 4, 2048, 16, 4, 128
NCORES = 8
HPC = H // NCORES          # q heads per core
JOBS = B * HPC             # jobs per core
NKT = S // P               # k tiles per row (16)
NQB = S // 512             # q blocks of 512 (4)
SCALE = 0.08838834764831845

f32 = mybir.dt.float32
f32r = mybir.dt.float32r
bf16 = mybir.dt.bfloat16

_NC_CACHE = {}


def _build():
    nc = bacc.Bacc(None, target_bir_lowering=False)
    qt = nc.dram_tensor("qt", [JOBS, P, S], f32r, kind="ExternalInput")
    kt = nc.dram_tensor("kt", [B, P, S], f32r, kind="ExternalInput")
    va = nc.dram_tensor("va", [B, P, NKT, P + 1], bf16, kind="ExternalInput")
    o = nc.dram_tensor("o", [JOBS, S, D], f32, kind="ExternalOutput")

    tri_np = np.triu(np.ones((P, P), dtype=np.float32)).astype(ml_dtypes.bfloat16)
    tri_dram = nc.inline_tensor(tri_np, name="tri")

    with tile.TileContext(nc) as tc:
        with (
            tc.tile_pool(name="cons", bufs=1) as cons,
            tc.tile_pool(name="kv", bufs=2) as kv,
            tc.tile_pool(name="qp", bufs=2) as qp,
            tc.tile_pool(name="pp", bufs=4) as pp,
            tc.tile_pool(name="op", bufs=4) as op,
            tc.tile_pool(name="ps2", bufs=2, space="PSUM") as ps2,
            tc.tile_pool(name="pso", bufs=2, space="PSUM") as pso,
        ):
            trim = cons.tile([P, P], bf16, tag="tri")
            nc.sync.dma_start(trim[:], tri_dram[:])

            for b in range(B):
                kt_sb = kv.tile([P, S], f32r, tag="kt", name=f"kt{b}")
                va_sb = kv.tile([P, NKT, P + 1], bf16, tag="va", name=f"va{b}")
                nc.sync.dma_start(kt_sb[:], kt[b])
                nc.sync.dma_start(va_sb[:], va[b])

                for h in range(HPC):
                    job = b * HPC + h
                    qt_sb = qp.tile([P, S], f32r, tag="qt", name=f"qt{job}")
                    nc.sync.dma_start(qt_sb[:], qt[job])

                    for qb in range(NQB):
                        # two PSUM banks, each holding two [q, V|l] accumulators
                        po = [
                            pso.tile([P, 2, P + 2], f32, tag=f"po{j}", name=f"po_{job}_{qb}_{j}")
                            for j in range(2)
                        ]

                        def pv(p_sb, pcol, qj, ko):
                            nc.tensor.matmul(
                                po[qj // 2][:, qj % 2, 0:P + 1],
                                p_sb[:, pcol:pcol + P],
                                va_sb[:, ko, :],
                                start=(ko == 0 and qj % 2 == 0),
                                stop=(ko == 4 * qb + qj),
                                skip_group_check=True,
                            )

                        # full (below-diagonal) k-tiles, two at a time
                        for kc in range(2 * qb):
                            s_ps = ps2.tile([P, 1024], f32, tag="s2", name=f"s2_{job}_{qb}_{kc}")
                            for i in range(2):
                                nc.tensor.matmul(
                                    s_ps[:, i * 512:(i + 1) * 512],
                                    kt_sb[:, (2 * kc + i) * P:(2 * kc + i + 1) * P],
                                    qt_sb[:, qb * 512:(qb + 1) * 512],
                                    start=True, stop=True,
                                )
                            p_sb = pp.tile([P, 1024], bf16, tag="p2", name=f"p2_{job}_{qb}_{kc}")
                            nc.scalar.activation(
                                p_sb[:], s_ps[:],
                                mybir.ActivationFunctionType.Exp, scale=SCALE,
                            )
                            for i in range(2):
                                for qj in range(4):
                                    pv(p_sb, i * 512 + qj * P, qj, 2 * kc + i)

                        # diagonal k-tile jd=0: full width + triangular mask
                        ko0 = 4 * qb
                        s_ps = ps2.tile([P, 1024], f32, tag="s2", name=f"sd0_{job}_{qb}")
                        nc.tensor.matmul(
                            s_ps[:, 0:512],
                            kt_sb[:, ko0 * P:(ko0 + 1) * P],
                            qt_sb[:, qb * 512:(qb + 1) * 512],
                            start=True, stop=True,
                        )
                        p_sb = pp.tile([P, 1024], bf16, tag="p2", name=f"pd0_{job}_{qb}")
                        nc.scalar.activation(
                            p_sb[:, 0:512], s_ps[:, 0:512],
                            mybir.ActivationFunctionType.Exp, scale=SCALE,
                        )
                        nc.vector.tensor_mul(p_sb[:, 0:P], p_sb[:, 0:P], trim[:])
                        for qj in range(4):
                            pv(p_sb, qj * P, qj, ko0)

                        # diagonal k-tiles jd=1..3, packed into one 2-bank
                        # region: jd1 -> [0:384], jd2 -> [512:768],
                        # jd3 -> [768:896]; one exp spans [0:896] (cols
                        # [384:512] are stale garbage, never consumed)
                        s_ps = ps2.tile([P, 1024], f32, tag="s2", name=f"sd123_{job}_{qb}")
                        offs = {1: -128, 2: 256, 3: 384}  # flat = qcol + off
                        for jd in range(1, 4):
                            ko = 4 * qb + jd
                            q0 = jd * P
                            nc.tensor.matmul(
                                s_ps[:, q0 + offs[jd]:512 + offs[jd]],
                                kt_sb[:, ko * P:(ko + 1) * P],
                                qt_sb[:, qb * 512 + q0:(qb + 1) * 512],
                                start=True, stop=True,
                            )
                        p_sb = pp.tile([P, 1024], bf16, tag="p2", name=f"pd123_{job}_{qb}")
                        nc.scalar.activation(
                            p_sb[:, 0:896], s_ps[:, 0:896],
                            mybir.ActivationFunctionType.Exp, scale=SCALE,
                        )
                        for jd in range(1, 4):
                            ko = 4 * qb + jd
                            dcol = jd * P + offs[jd]
                            nc.vector.tensor_mul(
                                p_sb[:, dcol:dcol + P], p_sb[:, dcol:dcol + P], trim[:],
                            )
                            for qj in range(jd, 4):
                                pv(p_sb, qj * P + offs[jd], qj, ko)

                        # normalize + store, one pass per PSUM bank (2 q tiles)
                        for j in range(2):
                            rec = op.tile([P, 2], f32, tag="rec", name=f"rec_{job}_{qb}_{j}")
                            nc.vector.reciprocal(rec[:], po[j][:, :, P])
                            o_sb = op.tile([P, 2, P], f32, tag="o", name=f"o_{job}_{qb}_{j}")
                            nc.vector.tensor_tensor(
                                o_sb[:],
                                po[j][:, :, 0:P],
                                rec[:, :, None].to_broadcast([P, 2, P]),
                                mybir.AluOpType.mult,
                            )
                            r0 = (qb * 4 + 2 * j) * P
                            nc.sync.dma_start(
                                o[job, r0:r0 + 2 * P, :].rearrange("(p q) d -> q p d", p=2),
                                o_sb[:],
                            )
    nc.compile()
    return nc


def _get_nc():
    if "nc" not in _NC_CACHE:
        _NC_CACHE["nc"] = _build()
    return _NC_CACHE["nc"]


def kernel(q, k, v, cu_seqlens=None, _trace=False):
    q = np.ascontiguousarray(q, dtype=np.float32).reshape(B, S, H, D)
    k = np.ascontiguousarray(k, dtype=np.float32).reshape(B, S, KH, D)
    v = np.ascontiguousarray(v, dtype=np.float32).reshape(B, S, KH, D)

    ones = np.ones((B, S, KH, 1), np.float32)
    vaug = np.concatenate([v, ones], axis=3)          # [B, S, KH, 129]
    # [B, S, KH, 129] -> [KH, B, kp, ko, 129]
    vaug = vaug.reshape(B, NKT, P, KH, P + 1).transpose(3, 0, 2, 1, 4)
    vaug = np.ascontiguousarray(vaug.astype(ml_dtypes.bfloat16))
    # k: [B, S, KH, D] -> [KH, B, D, S]
    ktr = np.ascontiguousarray(k.transpose(2, 0, 3, 1))

    in_maps = []
    for c in range(NCORES):
        g = (c * HPC) // (H // KH)   # kv head for this core
        qc = q[:, :, c * HPC:(c + 1) * HPC, :]        # [B, S, HPC, D]
        qtr = qc.transpose(0, 2, 3, 1).reshape(JOBS, D, S)  # [(b h), D, S]
        in_maps.append({
            "qt": np.ascontiguousarray(qtr),
            "kt": ktr[g],
            "va": vaug[g],
        })

    nc = _get_nc()
    res = run_bass_kernel_spmd(nc, in_maps, list(range(NCORES)), trace=_trace)

    out = np.empty((B, S, H, D), dtype=np.float32)
    for c in range(NCORES):
        oc = res.results[c]["o"].reshape(B, HPC, S, D)
        out[:, :, c * HPC:(c + 1) * HPC, :] = oc.transpose(0, 2, 1, 3)
    out = out.reshape(B * S, H, D)
    if _trace:
        return out, res
    return out


# revision 9
# speedup vs baseline: 1.2297x; 1.2297x over previous
"""Causal GQA attention (B=4, S=2048, H=16, KH=4, D=128) on 8 TRN2 NeuronCores.

Sharding: tensor-parallel over heads — each core owns 2 query heads and the
matching KV head; per core that is 8 independent (batch, head) causal
attention jobs of S=2048, D=128.

Per-core kernel (Bass/Tile), per job:
  - scores computed transposed, sT[k, q] = K_tile^T-stationary matmul against
    pre-transposed Q (f32r, full PE rate at N>=256)
  - softmax without running max (scores bounded: |SCALE * qk| < ~10), so
    P = exp(SCALE * sT) directly on ScalarE (PSUM -> SBUF bf16); full
    (below-diagonal) k-tiles are exp'd two at a time from a 2-bank PSUM
    region to amortize ScalarE instruction overhead
  - causal mask applied post-exp as a bf16 triangular multiply on the
    diagonal 128x128 tile only
  - PV: P-tile-stationary matmul against V augmented with a ones column;
    PSUM accumulates both the output numerator and the softmax denominator
    across k tiles; two q-tiles' accumulators share one PSUM bank.
    Final normalize = batched reciprocal + broadcast multiply on DVE.
"""

import numpy as np
import ml_dtypes

import concourse.bass as bass
import concourse.mybir as mybir
import concourse.tile as tile
from concourse import bacc
from concourse.bass_utils import run_bass_kernel_spmd

P = 128
B, S, H, KH, D = 4, 2048, 16, 4, 128
NCORES = 8
HPC = H // NCORES          # q heads per core
JOBS = B * HPC             # jobs per core
NKT = S // P               # k tiles per row (16)
NQB = S // 512             # q blocks of 512 (4)
SCALE = 0.08838834764831845

f32 = mybir.dt.float32
f32r = mybir.dt.float32r
bf16 = mybir.dt.bfloat16

_NC_CACHE = {}


def _build():
    nc = bacc.Bacc(None, target_bir_lowering=False)
    qt = nc.dram_tensor("qt", [JOBS, P, S], f32r, kind="ExternalInput")
    kt = nc.dram_tensor("kt", [B, P, S], f32r, kind="ExternalInput")
    va = nc.dram_tensor("va", [B, P, NKT, P + 1], bf16, kind="ExternalInput")
    o = nc.dram_tensor("o", [JOBS, S, D], f32, kind="ExternalOutput")

    tri_np = np.triu(np.ones((P, P), dtype=np.float32)).astype(ml_dtypes.bfloat16)
    tri_dram = nc.inline_tensor(tri_np, name="tri")

    with tile.TileContext(nc) as tc:
        with (
            tc.tile_pool(name="cons", bufs=1) as cons,
            tc.tile_pool(name="kv", bufs=2) as kv,
            tc.tile_pool(name="qp", bufs=2) as qp,
            tc.tile_pool(name="pp", bufs=4) as pp,
            tc.tile_pool(name="op", bufs=4) as op,
            tc.tile_pool(name="ps2", bufs=3, space="PSUM") as ps2,
            tc.tile_pool(name="pso", bufs=1, space="PSUM") as pso,
        ):
            trim = cons.tile([P, P], bf16, tag="tri")
            nc.sync.dma_start(trim[:], tri_dram[:])

            for b in range(B):
                kt_sb = kv.tile([P, S], f32r, tag="kt", name=f"kt{b}")
                va_sb = kv.tile([P, NKT, P + 1], bf16, tag="va", name=f"va{b}")
                nc.sync.dma_start(kt_sb[:], kt[b])
                nc.sync.dma_start(va_sb[:], va[b])

                for h in range(HPC):
                    job = b * HPC + h
                    qt_sb = qp.tile([P, S], f32r, tag="qt", name=f"qt{job}")
                    nc.sync.dma_start(qt_sb[:], qt[job])

                    for qb in range(NQB):
                        # two PSUM banks, each holding two [q, V|l] accumulators
                        po = [
                            pso.tile([P, 2, P + 2], f32, tag=f"po{j}", name=f"po_{job}_{qb}_{j}")
                            for j in range(2)
                        ]

                        def pv(p_sb, pcol, qj, ko):
                            nc.tensor.matmul(
                                po[qj // 2][:, qj % 2, 0:P + 1],
                                p_sb[:, pcol:pcol + P],
                                va_sb[:, ko, :],
                                start=(ko == 0 and qj % 2 == 0),
                                stop=(ko == 4 * qb + qj),
                                skip_group_check=True,
                            )

                        # full (below-diagonal) k-tiles, two at a time
                        for kc in range(2 * qb):
                            s_ps = ps2.tile([P, 1024], f32, tag="s2", name=f"s2_{job}_{qb}_{kc}")
                            for i in range(2):
                                nc.tensor.matmul(
                                    s_ps[:, i * 512:(i + 1) * 512],
                                    kt_sb[:, (2 * kc + i) * P:(2 * kc + i + 1) * P],
                                    qt_sb[:, qb * 512:(qb + 1) * 512],
                                    start=True, stop=True,
                                )
                            p_sb = pp.tile([P, 1024], bf16, tag="p2", name=f"p2_{job}_{qb}_{kc}")
                            nc.scalar.activation(
                                p_sb[:], s_ps[:],
                                mybir.ActivationFunctionType.Exp, scale=SCALE,
                            )
                            for i in range(2):
                                for qj in range(4):
                                    pv(p_sb, i * 512 + qj * P, qj, 2 * kc + i)

                        # diagonal k-tile jd=0: full width + triangular mask
                        ko0 = 4 * qb
                        s_ps = ps2.tile([P, 1024], f32, tag="s2", name=f"sd0_{job}_{qb}")
                        nc.tensor.matmul(
                            s_ps[:, 0:512],
                            kt_sb[:, ko0 * P:(ko0 + 1) * P],
                            qt_sb[:, qb * 512:(qb + 1) * 512],
                            start=True, stop=True,
                        )
                        p_sb = pp.tile([P, 1024], bf16, tag="p2", name=f"pd0_{job}_{qb}")
                        nc.scalar.activation(
                            p_sb[:, 0:512], s_ps[:, 0:512],
                            mybir.ActivationFunctionType.Exp, scale=SCALE,
                        )
                        nc.vector.tensor_mul(p_sb[:, 0:P], p_sb[:, 0:P], trim[:])
                        for qj in range(4):
                            pv(p_sb, qj * P, qj, ko0)

                        # diagonal k-tiles jd=1..3, packed into one 2-bank
                        # region: jd1 -> [0:384], jd2 -> [512:768],
                        # jd3 -> [768:896]; one exp spans [0:896] (cols
                        # [384:512] are stale garbage, never consumed)
                        s_ps = ps2.tile([P, 1024], f32, tag="s2", name=f"sd123_{job}_{qb}")
                        offs = {1: -128, 2: 256, 3: 384}  # flat = qcol + off
                        for jd in range(1, 4):
                            ko = 4 * qb + jd
                            q0 = jd * P
                            nc.tensor.matmul(
                                s_ps[:, q0 + offs[jd]:512 + offs[jd]],
                                kt_sb[:, ko * P:(ko + 1) * P],
                                qt_sb[:, qb * 512 + q0:(qb + 1) * 512],
                                start=True, stop=True,
                            )
                        p_sb = pp.tile([P, 1024], bf16, tag="p2", name=f"pd123_{job}_{qb}")
                        nc.scalar.activation(
                            p_sb[:, 0:896], s_ps[:, 0:896],
                            mybir.ActivationFunctionType.Exp, scale=SCALE,
                        )
                        for jd in range(1, 4):
                            ko = 4 * qb + jd
                            dcol = jd * P + offs[jd]
                            nc.vector.tensor_mul(
                                p_sb[:, dcol:dcol + P], p_sb[:, dcol:dcol + P], trim[:],
                            )
                            for qj in range(jd, 4):
                                pv(p_sb, qj * P + offs[jd], qj, ko)

                        # normalize + store, one pass per PSUM bank (2 q tiles)
                        for j in range(2):
                            rec = op.tile([P, 2], f32, tag="rec", name=f"rec_{job}_{qb}_{j}")
                            nc.vector.reciprocal(rec[:], po[j][:, :, P])
                            o_sb = op.tile([P, 2, P], f32, tag="o", name=f"o_{job}_{qb}_{j}")
                            nc.vector.tensor_tensor(
                                o_sb[:],
                                po[j][:, :, 0:P],
                                rec[:, :, None].to_broadcast([P, 2, P]),
                                mybir.AluOpType.mult,
                            )
                            r0 = (qb * 4 + 2 * j) * P
                            nc.sync.dma_start(
                                o[job, r0:r0 + 2 * P, :].rearrange("(p q) d -> q p d", p=2),
                                o_sb[:],
                            )
    nc.compile()
    return nc


def _get_nc():
    if "nc" not in _NC_CACHE:
        _NC_CACHE["nc"] = _build()
    return _NC_CACHE["nc"]


def kernel(q, k, v, cu_seqlens=None, _trace=False):
    q = np.ascontiguousarray(q, dtype=np.float32).reshape(B, S, H, D)
    k = np.ascontiguousarray(k, dtype=np.float32).reshape(B, S, KH, D)
    v = np.ascontiguousarray(v, dtype=np.float32).reshape(B, S, KH, D)

    ones = np.ones((B, S, KH, 1), np.float32)
    vaug = np.concatenate([v, ones], axis=3)          # [B, S, KH, 129]
    # [B, S, KH, 129] -> [KH, B, kp, ko, 129]
    vaug = vaug.reshape(B, NKT, P, KH, P + 1).transpose(3, 0, 2, 1, 4)
    vaug = np.ascontiguousarray(vaug.astype(ml_dtypes.bfloat16))
    # k: [B, S, KH, D] -> [KH, B, D, S]
    ktr = np.ascontiguousarray(k.transpose(2, 0, 3, 1))

    in_maps = []
    for c in range(NCORES):
        g = (c * HPC) // (H // KH)   # kv head for this core
        qc = q[:, :, c * HPC:(c + 1) * HPC, :]        # [B, S, HPC, D]
        qtr = qc.transpose(0, 2, 3, 1).reshape(JOBS, D, S)  # [(b h), D, S]
        in_maps.append({
            "qt": np.ascontiguousarray(qtr),
            "kt": ktr[g],
            "va": vaug[g],
        })

    nc = _get_nc()
    res = run_bass_kernel_spmd(nc, in_maps, list(range(NCORES)), trace=_trace)

    out = np.empty((B, S, H, D), dtype=np.float32)
    for c in range(NCORES):
        oc = res.results[c]["o"].reshape(B, HPC, S, D)
        out[:, :, c * HPC:(c + 1) * HPC, :] = oc.transpose(0, 2, 1, 3)
    out = out.reshape(B * S, H, D)
    if _trace:
        return out, res
    return out
